# revision 17
# baseline (speedup 1.0000x reference)
"""TRN2 Bass/Tile kernel for nn_Block_89842125898023 (dense transformer
block), SPMD over 8 NeuronCores.

Sharding (data-parallel over batch x query-halves, zero collectives):
core c handles batch element b = c//2 and query half p = c%2 of that
element's 2048 tokens, using a "zigzag" split (p=0: tokens [0,512) u
[1536,2048); p=1: [512,1536)) so the causal-attention work is identical
on every core. Each core redundantly computes K/V for its batch
element's full sequence from the (replicated) xT input.

v2 layout notes (vs the earlier DRAM-bounce version):
- K/Q/V and the attention output stay SBUF-resident end to end; the
  only DRAM traffic is inputs, weights and the final output.
- All weights are bf16 and streamed with plain contiguous HWDGE DMAs
  ([128, >=512] tiles); no software-DGE rearranged gathers.
- proj/FFN run once over both query halves (1024 columns), so Wp/W1/W2
  are each streamed exactly once per core.
- Causal masking is applied multiplicatively post-exp on the gpsimd
  engine (DVE is loaded with LN/eviction work).
- On-device layout is feature-major ([C, T], channels on partitions);
  scores are computed transposed per head; the softmax normalizer comes
  from a ones-column appended to V (M=65 AV matmul); division by Z is
  deferred to the AV eviction. All matmul accumulation is fp32 PSUM;
  residuals are carried in f32r, matmul operands in bf16.

kernel(**inputs) takes the full unsharded inputs, builds per-core input
maps host-side, runs the SPMD program on cores 0-7 via
bass_utils.run_bass_kernel_spmd, and reassembles the full output.
"""

import sys

sys.path.insert(0, "/opt/trn_rl_repo")

from contextlib import ExitStack

import numpy as np
import ml_dtypes

import concourse.bass as bass
import concourse.bacc as bacc
import concourse.tile as tile
from concourse import mybir
from concourse.bass_utils import run_bass_kernel_spmd

F32 = mybir.dt.float32
F32R = mybir.dt.float32r
BF16 = mybir.dt.bfloat16
AF = mybir.ActivationFunctionType
ALU = mybir.AluOpType
P = 128


class Cfg:
    def __init__(self, C=1024, H=16, D=64, Tkv=2048, eps=1e-5, ffn_mult=4):
        self.C = C
        self.H = H
        self.D = D
        assert H * D == C
        self.Tkv = Tkv
        self.Tq = Tkv // 2
        self.F = ffn_mult * C
        self.eps = eps
        self.NC = C // 128
        self.NF = self.F // 128
        self.NS = Tkv // 128
        self.scale = C ** -0.5
        self.TH = self.Tq // 2
        NS2 = self.NS // 2
        self.MB = 128 * (NS2 - 1) + self.TH
        self.MLO = ((Tkv - 128) - 128 * (NS2 - 1), 0)


def build_kernel(nc: bass.Bass, cfg: Cfg, ln_affine=True):
    c = cfg
    NH = c.C // 64  # head-halves
    TH = c.TH
    NS2 = c.NS // 2
    NTQ = c.Tq // 512  # 512-col chunks over the query axis

    xT_d = nc.dram_tensor("xT", [c.C, c.Tkv], BF16, kind="ExternalInput")
    xqTb_d = nc.dram_tensor("xqTb", [c.C, c.Tq], BF16, kind="ExternalInput")
    xqT_d = nc.dram_tensor("xqT", [c.C, c.Tq], F32R, kind="ExternalInput")
    wqkv_d = nc.dram_tensor("wqkv", [c.C, 3 * c.C], BF16,
                            kind="ExternalInput")
    wp_d = nc.dram_tensor("wp", [c.C, c.C], BF16, kind="ExternalInput")
    w1_d = nc.dram_tensor("w1", [c.C, c.F], BF16, kind="ExternalInput")
    w2_d = nc.dram_tensor("w2", [c.F, c.C], BF16, kind="ExternalInput")
    NV = 6 * (c.C // P) + c.F // P
    vecs_d = nc.dram_tensor("vecs", [P, NV], F32, kind="ExternalInput")
    mask_d = nc.dram_tensor("maskband", [P, 2 * c.MB], BF16,
                            kind="ExternalInput")
    out_d = nc.dram_tensor("outT", [c.C, c.Tq], F32, kind="ExternalOutput")

    with ExitStack() as ctx:
        tc = ctx.enter_context(tile.TileContext(nc))

        const_pool = ctx.enter_context(tc.tile_pool(name="const", bufs=1))
        ones_t = const_pool.tile([P, 1], F32)
        nc.vector.memset(ones_t[:], 1.0)
        zerob = const_pool.tile([P, 1], F32, name="zerob")
        nc.vector.memset(zerob[:], 0.0)
        epsb = const_pool.tile([1, 1], F32, name="epsb")
        nc.vector.memset(epsb[:], float(c.eps))
        ones_bf = const_pool.tile([P, 1], BF16, name="ones_bf")
        nc.vector.memset(ones_bf[:], 1.0)
        ones_r = const_pool.tile([P, 1], F32R, name="ones_r")
        nc.vector.tensor_copy(ones_r[:], ones_t[:])

        vec_tile = const_pool.tile([P, NV], F32, name="vecs")
        nc.sync.dma_start(out=vec_tile[:], in_=vecs_d.ap())
        _vo = [0]

        def vec_cols(n):
            k = n // P
            cols = [vec_tile[:, _vo[0] + i:_vo[0] + i + 1] for i in range(k)]
            _vo[0] += k
            return cols

        ln1g, ln1b = vec_cols(c.C), vec_cols(c.C)
        ln2g, ln2b = vec_cols(c.C), vec_cols(c.C)
        bp, b1, b2 = vec_cols(c.C), vec_cols(c.F), vec_cols(c.C)

        # attention output (concat heads, feature-major), both halves
        attn_pool = ctx.enter_context(tc.tile_pool(name="attn", bufs=1))
        attn_tiles = [attn_pool.tile([P, c.Tq], BF16, name=f"at{i}")
                      for i in range(c.NC)]

        with ExitStack() as pab:
            k_pool = pab.enter_context(tc.tile_pool(name="k", bufs=1))
            k_tiles = [k_pool.tile([P, c.Tkv], BF16, name=f"k{i}")
                       for i in range(c.NC)]
            q_pool = pab.enter_context(tc.tile_pool(name="q", bufs=1))
            q_tiles = [q_pool.tile([P, c.Tq], BF16, name=f"q{i}")
                       for i in range(c.NC)]
            v_pool = pab.enter_context(tc.tile_pool(name="v", bufs=1))
            v_tiles = [v_pool.tile([P, NH, 65], BF16, name=f"v{s}")
                       for s in range(c.NS)]

            # ---------- phase A+B: LN1, Q/K/V, attention ----------
            # V[8..15] is interleaved with w=0 attention per head so the
            # activation engine's exp work overlaps PE's V matmuls.
            with ExitStack() as pa:
                x_pool = pa.enter_context(tc.tile_pool(name="xT", bufs=1))
                xq_pool = pa.enter_context(tc.tile_pool(name="xq", bufs=1))
                w_pool = pa.enter_context(tc.tile_pool(name="wqkv", bufs=1))

                xq_tiles = []
                for ci in range(c.NC):
                    t = xq_pool.tile([P, c.Tq], BF16, name=f"xq{ci}")
                    nc.sync.dma_start(
                        out=t[:], in_=xqTb_d.ap()[ci * P:(ci + 1) * P, :])
                    xq_tiles.append(t)
                x_tiles = []
                for ci in range(c.NC):
                    t = x_pool.tile([P, c.Tkv], BF16, name=f"x{ci}")
                    nc.sync.dma_start(out=t[:],
                                      in_=xT_d.ap()[ci * P:(ci + 1) * P, :])
                    x_tiles.append(t)

                # LN1 in place (x tiles only feed the projections)
                _layernorm_fm(nc, tc, c, xq_tiles, xq_tiles, c.Tq,
                              ln1g if ln_affine else None, ln1b,
                              ones_bf, zerob, epsb, "ln1q")
                _layernorm_fm(nc, tc, c, x_tiles, x_tiles, c.Tkv,
                              ln1g if ln_affine else None, ln1b,
                              ones_bf, zerob, epsb, "ln1")

                def load_w(which):
                    ts = []
                    for ci in range(c.NC):
                        wt = w_pool.tile([P, c.C], BF16, name=f"w{ci}")
                        nc.sync.dma_start(
                            out=wt[:],
                            in_=wqkv_d.ap()[ci * P:(ci + 1) * P,
                                            which * c.C:(which + 1) * c.C])
                        ts.append(wt)
                    return ts

                def v_block(s, psum_pool):
                    nc.vector.memset(v_tiles[s][:, :, 64:65], 1.0)
                    psv = [psum_pool.tile([P, 512], F32, name=f"pp{nf}")
                           for nf in range(2)]
                    for ci in range(c.NC):
                        for nf in range(2):
                            nc.tensor.matmul(
                                psv[nf][:],
                                lhsT=x_tiles[ci][:, s * P:(s + 1) * P],
                                rhs=wv_tiles[ci][:, nf * 512:(nf + 1) * 512],
                                start=(ci == 0), stop=(ci == c.NC - 1))
                    for nf in range(2):
                        nc.vector.tensor_copy(
                            v_tiles[s][:, nf * 8:(nf + 1) * 8, 0:64],
                            psv[nf][:].rearrange("p (h d) -> p h d", d=64))

                with ExitStack() as pq:
                    mm_psum = pq.enter_context(
                        tc.tile_pool(name="mm_psum", bufs=2, space="PSUM"))
                    # Q projection (query tokens only)
                    wq_tiles = load_w(0)
                    for fi in range(c.NC):
                        pss = [mm_psum.tile([P, 512], F32, name=f"pp{t}")
                               for t in range(NTQ)]
                        for ci in range(c.NC):
                            for t in range(NTQ):
                                nc.tensor.matmul(
                                    pss[t][:],
                                    lhsT=wq_tiles[ci][:,
                                                      fi * P:(fi + 1) * P],
                                    rhs=xq_tiles[ci][:,
                                                     t * 512:(t + 1) * 512],
                                    start=(ci == 0), stop=(ci == c.NC - 1))
                        for t in range(NTQ):
                            nc.vector.tensor_copy(
                                q_tiles[fi][:, t * 512:(t + 1) * 512],
                                pss[t][:])

                    # K projection (full sequence)
                    wk_tiles = load_w(1)
                    NTK = c.Tkv // 512
                    for fi in range(c.NC):
                        for t0 in range(0, NTK, 2):
                            pss = [mm_psum.tile([P, 512], F32,
                                                name=f"pp{t - t0}")
                                   for t in range(t0, t0 + 2)]
                            for ci in range(c.NC):
                                for i, t in enumerate(range(t0, t0 + 2)):
                                    nc.tensor.matmul(
                                        pss[i][:],
                                        lhsT=wk_tiles[ci][:,
                                                          fi * P:
                                                          (fi + 1) * P],
                                        rhs=x_tiles[ci][:,
                                                        t * 512:
                                                        (t + 1) * 512],
                                        start=(ci == 0),
                                        stop=(ci == c.NC - 1))
                            for i, t in enumerate(range(t0, t0 + 2)):
                                nc.vector.tensor_copy(
                                    k_tiles[fi][:, t * 512:(t + 1) * 512],
                                    pss[i][:])

                    # V projection, first half of the key blocks
                    wv_tiles = load_w(2)
                    for s in range(NS2):
                        v_block(s, mm_psum)

                # attention pools (mm_psum freed: 2+4+2 = 8 PSUM banks)
                sc_psum = pa.enter_context(
                    tc.tile_pool(name="sc_psum", bufs=2, space="PSUM"))
                av_psum = pa.enter_context(
                    tc.tile_pool(name="av_psum", bufs=1, space="PSUM"))
                vb_psum = pa.enter_context(
                    tc.tile_pool(name="vb_psum", bufs=1, space="PSUM"))
                e_pool = pa.enter_context(tc.tile_pool(name="e", bufs=3))
                r_pool = pa.enter_context(tc.tile_pool(name="r", bufs=2))
                mk_pool = pa.enter_context(tc.tile_pool(name="mk", bufs=1))

                mask_t = mk_pool.tile([P, 2 * c.MB], BF16, name="maskband")
                nc.sync.dma_start(out=mask_t[:], in_=mask_d.ap())

                def attn_head(w, hp):
                    wsl = slice(w * TH, (w + 1) * TH)
                    avs = [av_psum.tile([65, TH], F32, name=f"av{half}")
                           for half in range(2)]
                    NJ = NS2 if w == 0 else c.NS
                    # j in pairs: scores for (j, j+1) land in the two
                    # halves of one 2-bank PSUM tile, one Exp covers both
                    for j0 in range(0, NJ, 2):
                        for half in range(2):
                            hsl = slice(half * 64, half * 64 + 64)
                            head = 2 * hp + half
                            ps = sc_psum.tile([P, 2 * TH], F32,
                                              name="ps_sc")
                            for i, j in enumerate((j0, j0 + 1)):
                                nc.tensor.matmul(
                                    ps[:, i * TH:(i + 1) * TH],
                                    lhsT=k_tiles[hp][hsl,
                                                     j * P:(j + 1) * P],
                                    rhs=q_tiles[hp][hsl, wsl],
                                    start=True, stop=True)
                            et = e_pool.tile([P, 2 * TH], BF16, name="et")
                            nc.scalar.activation(et[:], ps[:], AF.Exp,
                                                 bias=zerob[:],
                                                 scale=float(c.scale))
                            for i, j in enumerate((j0, j0 + 1)):
                                if w == 0 or j >= NS2:
                                    cj = (w * c.MB + (c.Tkv - 128)
                                          - 128 * j - c.MLO[w])
                                    esl = slice(i * TH, (i + 1) * TH)
                                    nc.vector.tensor_tensor(
                                        et[:, esl], et[:, esl],
                                        mask_t[:, cj: cj + TH],
                                        op=ALU.mult)
                            for i, j in enumerate((j0, j0 + 1)):
                                nc.tensor.matmul(
                                    avs[half][:],
                                    lhsT=v_tiles[j][:, head, :],
                                    rhs=et[:, i * TH:(i + 1) * TH],
                                    start=(j == 0),
                                    stop=(j == NJ - 1))
                    for half in range(2):
                        av = avs[half]
                        rt0 = r_pool.tile([1, TH], F32, name="rt0")
                        nc.vector.reciprocal(rt0[:], av[64:65, :])
                        rb = r_pool.tile([64, TH], F32, name="rb")
                        nc.gpsimd.partition_broadcast(rb[:], rt0[:])
                        if half == 0:
                            nc.vector.tensor_tensor(
                                attn_tiles[hp][0:64, wsl],
                                av[0:64, :], rb[:], op=ALU.mult)
                        else:
                            # engines can't shift partitions; bounce
                            # through SBUF->SBUF DMA to rows 64-127
                            ab = r_pool.tile([64, TH], BF16, name="ab")
                            nc.vector.tensor_tensor(
                                ab[:], av[0:64, :], rb[:], op=ALU.mult)
                            nc.sync.dma_start(
                                out=attn_tiles[hp][64:128, wsl],
                                in_=ab[:])

                # w=0 attention only needs v[0..NS2); interleave the
                # remaining V blocks with it to keep the PE fed while
                # the activation engine chews the exps
                for hp in range(c.NC):
                    v_block(NS2 + hp, vb_psum)
                    attn_head(0, hp)
                for hp in range(c.NC):
                    attn_head(1, hp)

        # ---------- phase C: proj + residual (both halves) ----------
        with ExitStack() as pcd:
            x1_pool = pcd.enter_context(tc.tile_pool(name="x1", bufs=1))
            x1_tiles = [x1_pool.tile([P, c.Tq], F32R, name=f"x1{i}")
                        for i in range(c.NC)]
            xs_pool = pcd.enter_context(tc.tile_pool(name="xs", bufs=2))
            with ExitStack() as pc:
                wp_pool = pc.enter_context(tc.tile_pool(name="wp", bufs=1))
                pj_psum = pc.enter_context(
                    tc.tile_pool(name="pj_psum", bufs=1, space="PSUM"))

                wp_tiles = []
                for ci in range(c.NC):
                    wt = wp_pool.tile([P, c.C], BF16, name=f"wp{ci}")
                    nc.sync.dma_start(
                        out=wt[:], in_=wp_d.ap()[ci * P:(ci + 1) * P, :])
                    wp_tiles.append(wt)

                for fg0 in range(0, c.NC, 4):
                    fis = range(fg0, fg0 + 4)
                    pss = {(fi, t): pj_psum.tile(
                        [P, 512], F32, name=f"pj{fi - fg0}_{t}")
                        for fi in fis for t in range(NTQ)}
                    for ci in range(c.NC):
                        for fi in fis:
                            for t in range(NTQ):
                                nc.tensor.matmul(
                                    pss[(fi, t)][:],
                                    lhsT=wp_tiles[ci][:, fi * P:(fi + 1) * P],
                                    rhs=attn_tiles[ci][:,
                                                       t * 512:(t + 1) * 512],
                                    start=(ci == 0), stop=(ci == c.NC - 1))
                    for fi in fis:
                        xqs = xs_pool.tile([P, c.Tq], F32R, name="xqs")
                        nc.sync.dma_start(
                            out=xqs[:], in_=xqT_d.ap()[fi * P:(fi + 1) * P, :])
                        for t in range(NTQ):
                            tsl = slice(t * 512, (t + 1) * 512)
                            ev = xs_pool.tile([P, 512], F32, name="ev")
                            nc.vector.tensor_scalar(ev[:], pss[(fi, t)][:],
                                                    bp[fi][:], None,
                                                    op0=ALU.add)
                            nc.vector.tensor_tensor(
                                x1_tiles[fi][:, tsl], ev[:], xqs[:, tsl],
                                op=ALU.add)

            # ---------- phase D: LN2 + FFN (both halves) ----------
            with ExitStack() as pd:
                h2_pool = pd.enter_context(tc.tile_pool(name="h2", bufs=1))
                h2_tiles = [h2_pool.tile([P, c.Tq], BF16, name=f"h2_{i}")
                            for i in range(c.NC)]
                _layernorm_fm(nc, tc, c, x1_tiles, h2_tiles, c.Tq,
                              ln2g if ln_affine else None, ln2b,
                              ones_r, zerob, epsb, "ln2")

                relu_pool = pd.enter_context(tc.tile_pool(name="relu",
                                                          bufs=1))
                relu_tiles = [relu_pool.tile([P, c.Tq], BF16, name=f"r{i}")
                              for i in range(c.NF)]

                # FFN W1: stream w1 once, both halves as rhs columns
                with ExitStack() as pw1:
                    w1_pool = pw1.enter_context(
                        tc.tile_pool(name="w1s", bufs=2))
                    ff_psum = pw1.enter_context(
                        tc.tile_pool(name="ff_psum", bufs=2, space="PSUM"))
                    for fg in range(c.F // c.C):
                        w1_tiles = []
                        for ci in range(c.NC):
                            wt = w1_pool.tile([P, c.C], BF16,
                                              name=f"w1s{ci}")
                            nc.sync.dma_start(
                                out=wt[:],
                                in_=w1_d.ap()[ci * P:(ci + 1) * P,
                                              fg * c.C:(fg + 1) * c.C])
                            w1_tiles.append(wt)
                        for fsub in range(c.NC):
                            f = fg * c.NC + fsub
                            psw = [ff_psum.tile([P, 512], F32,
                                                name=f"pw{t}")
                                   for t in range(NTQ)]
                            for ci in range(c.NC):
                                for t in range(NTQ):
                                    nc.tensor.matmul(
                                        psw[t][:],
                                        lhsT=w1_tiles[ci][:,
                                                          fsub * P:
                                                          (fsub + 1) * P],
                                        rhs=h2_tiles[ci][:,
                                                         t * 512:
                                                         (t + 1) * 512],
                                        start=(ci == 0),
                                        stop=(ci == c.NC - 1))
                            for t in range(NTQ):
                                nc.scalar.activation(
                                    relu_tiles[f][:, t * 512:(t + 1) * 512],
                                    psw[t][:], AF.Relu, bias=b1[f][:])

                # FFN W2 + residual + out: fi-halves so PSUM fits
                with ExitStack() as pw2:
                    w2_pool = pw2.enter_context(
                        tc.tile_pool(name="w2s", bufs=4))
                    w2_psum = pw2.enter_context(
                        tc.tile_pool(name="w2_psum", bufs=1, space="PSUM"))
                    for fih in range(2):
                        fis = range(fih * 4, fih * 4 + 4)
                        pss = {(fi, t): w2_psum.tile(
                            [P, 512], F32, name=f"p2{fi % 4}_{t}")
                            for fi in fis for t in range(NTQ)}
                        for fb in range(c.NF):
                            wt = w2_pool.tile([P, 512], BF16, name="w2t")
                            nc.sync.dma_start(
                                out=wt[:],
                                in_=w2_d.ap()[fb * P:(fb + 1) * P,
                                              fih * 512:(fih + 1) * 512])
                            for fi in fis:
                                fo = (fi - fih * 4) * P
                                for t in range(NTQ):
                                    nc.tensor.matmul(
                                        pss[(fi, t)][:],
                                        lhsT=wt[:, fo:fo + P],
                                        rhs=relu_tiles[fb][:,
                                                           t * 512:
                                                           (t + 1) * 512],
                                        start=(fb == 0),
                                        stop=(fb == c.NF - 1))
                        for fi in fis:
                            for t in range(NTQ):
                                tsl = slice(t * 512, (t + 1) * 512)
                                ev = xs_pool.tile([P, 512], F32, name="ev2")
                                nc.vector.tensor_scalar(
                                    ev[:], pss[(fi, t)][:], b2[fi][:], None,
                                    op0=ALU.add)
                                nc.vector.tensor_tensor(
                                    ev[:], ev[:], x1_tiles[fi][:, tsl],
                                    op=ALU.add)
                                nc.sync.dma_start(
                                    out=out_d.ap()[fi * P:(fi + 1) * P, tsl],
                                    in_=ev[:])
    return nc


def _layernorm_fm(nc, tc, c, x_tiles, out_tiles, T, g_tiles, b_tiles,
                  ones_t, zerob, epsb, name):
    """Feature-major layernorm: features on partitions, stats via
    ones-vector matmuls on the PE. Stats + apply are fused per 512-col
    chunk so row temporaries stay chunk-sized. In-place (out == x) is
    safe: chunk tt's stats read x[:, tt] strictly before its apply
    writes it, and other chunks touch disjoint columns."""
    with ExitStack() as ctx:
        TT = min(512, T)
        NT = T // TT
        sq_pool = ctx.enter_context(tc.tile_pool(name=f"{name}_sq", bufs=2))
        st_psum = ctx.enter_context(
            tc.tile_pool(name=f"{name}_stp", bufs=2, space="PSUM"))
        tmp_pool = ctx.enter_context(tc.tile_pool(name=f"{name}_tmp",
                                                  bufs=2))

        bf_in = x_tiles[0].dtype == BF16 and out_tiles[0].dtype == BF16
        bc_dt = BF16 if bf_in else F32
        for tt in range(NT):
            sl = slice(tt * TT, (tt + 1) * TT)
            ps1 = st_psum.tile([1, TT], F32, name="ps1")
            ps2 = st_psum.tile([1, TT], F32, name="ps2")
            sq_dt = x_tiles[0].dtype
            for ci, xt in enumerate(x_tiles):
                st, sp = ci == 0, ci == len(x_tiles) - 1
                nc.tensor.matmul(ps1[:], lhsT=ones_t[:],
                                 rhs=xt[:, sl], start=st, stop=sp)
                sq = sq_pool.tile([P, TT], sq_dt, name="sq")
                if sq_dt == BF16:
                    nc.vector.tensor_tensor(sq[:], xt[:, sl], xt[:, sl],
                                            op=ALU.mult)
                else:
                    nc.scalar.activation(sq[:], xt[:, sl], AF.Square,
                                         bias=zerob[:])
                nc.tensor.matmul(ps2[:], lhsT=ones_t[:], rhs=sq[:],
                                 start=st, stop=sp)
            mu = tmp_pool.tile([1, TT], F32, name="mu")
            nc.scalar.mul(mu[:], ps1[:], 1.0 / c.C)
            mu2 = tmp_pool.tile([1, TT], F32, name="mu2")
            nc.scalar.activation(mu2[:], mu[:], AF.Square, bias=zerob[0:1])
            var = tmp_pool.tile([1, TT], F32, name="var")
            nc.scalar.mul(var[:], ps2[:], 1.0 / c.C)
            nc.vector.tensor_sub(var[:], var[:], mu2[:])
            sd = tmp_pool.tile([1, TT], F32, name="sd")
            nc.scalar.activation(sd[:], var[:], AF.Sqrt, bias=epsb[:])
            rs = tmp_pool.tile([1, TT], F32, name="rs")
            nc.vector.reciprocal(rs[:], sd[:])
            nmrs = tmp_pool.tile([1, TT], F32, name="nmrs")
            nc.vector.tensor_tensor(nmrs[:], mu[:], rs[:], op=ALU.mult)
            nc.vector.tensor_scalar_mul(nmrs[:], nmrs[:], -1.0)

            if bf_in:
                rrow = tmp_pool.tile([1, TT], BF16, name="rrowb")
                nc.vector.tensor_copy(rrow[:], rs[:])
                nrow = tmp_pool.tile([1, TT], BF16, name="nrowb")
                nc.vector.tensor_copy(nrow[:], nmrs[:])
            else:
                rrow, nrow = rs, nmrs
            rs_b = tmp_pool.tile([P, TT], bc_dt, name="rsb")
            nmrs_b = tmp_pool.tile([P, TT], bc_dt, name="nmrsb")
            nc.gpsimd.partition_broadcast(rs_b[:], rrow[:])
            nc.gpsimd.partition_broadcast(nmrs_b[:], nrow[:])
            for ci, (xt, ot) in enumerate(zip(x_tiles, out_tiles)):
                nc.vector.tensor_tensor(ot[:, sl], xt[:, sl], rs_b[:],
                                        op=ALU.mult)
                nc.vector.tensor_tensor(ot[:, sl], ot[:, sl], nmrs_b[:],
                                        op=ALU.add)
                if g_tiles is not None:
                    nc.vector.tensor_scalar(ot[:, sl], ot[:, sl],
                                            g_tiles[ci][:], b_tiles[ci][:],
                                            op0=ALU.mult, op1=ALU.add)


# ======================= host side =======================

def round_f32r(x):
    u = np.asarray(x, np.float32).view(np.uint32).astype(np.uint64)
    lsb = (u >> np.uint64(12)) & np.uint64(1)
    u = u + (np.uint64(1) << np.uint64(11)) - np.uint64(1) + lsb
    u = (u >> np.uint64(12)) << np.uint64(12)
    return (u & np.uint64(0xFFFFFFFF)).astype(np.uint32).view(np.float32)


def zigzag_tokens(cfg, p):
    c = cfg
    if p == 0:
        return np.concatenate([np.arange(0, c.TH),
                               np.arange(c.Tkv - c.TH, c.Tkv)])
    return np.arange(c.TH, c.TH + c.Tq)


def _shared_inputs(cfg, inputs):
    """Per-call (not per-core) tensors: weights, vecs, per-p masks."""
    c = cfg
    bf = lambda a: np.ascontiguousarray(
        np.asarray(a).astype(ml_dtypes.bfloat16))

    def flat_w(w):
        return np.transpose(np.asarray(w), (1, 0, 2)).reshape(c.C, c.C)

    wqkv = bf(np.concatenate([flat_w(inputs["Wq"]), flat_w(inputs["Wk"]),
                              flat_w(inputs["Wv"])], axis=1))
    wp = bf(inputs["Wp"])
    w1 = bf(inputs["W1"])
    w2 = bf(inputs["W2"])

    vl = []
    for nm in ("ln1_g", "ln1_b", "ln2_g", "ln2_b", "bp", "b1", "b2"):
        v = np.asarray(inputs[nm], np.float32)
        vl.append(v.reshape(-1, 128).T)
    vecs = np.ascontiguousarray(np.concatenate(vl, axis=1))

    g = np.arange(c.MB)[None, :]
    pp = np.arange(128)[:, None]
    masks = []
    for p in range(2):
        qoff_w = ((0, c.Tkv - c.TH), (c.TH, c.Tq))[p]
        bands = [(g + c.MLO[w] >= pp + (c.Tkv - 128) - qoff_w[w])
                 .astype(np.float32) for w in range(2)]
        masks.append(bf(np.concatenate(bands, axis=1)))

    xTs = []
    for b in range(np.asarray(inputs["x"]).shape[0]):
        xTs.append(bf(np.asarray(inputs["x"])[b].T))
    return dict(wqkv=wqkv, wp=wp, w1=w1, w2=w2, vecs=vecs, masks=masks,
                xTs=xTs)


def host_prepare(cfg, inputs, core_id, shared):
    c = cfg
    b, p = core_id // 2, core_id % 2
    xb = np.asarray(inputs["x"])[b]
    qidx = zigzag_tokens(c, p)
    xq = np.ascontiguousarray(xb[qidx].T)
    return {
        "xT": shared["xTs"][b],
        "xqTb": xq.astype(ml_dtypes.bfloat16),
        "xqT": round_f32r(xq),
        "wqkv": shared["wqkv"], "wp": shared["wp"],
        "w1": shared["w1"], "w2": shared["w2"],
        "vecs": shared["vecs"],
        "maskband": shared["masks"][p],
    }


def host_gather(cfg, results, B):
    c = cfg
    out = np.empty((B, c.Tkv, c.C), np.float32)
    for core in range(2 * B):
        b, p = core // 2, core % 2
        out[b, zigzag_tokens(c, p), :] = np.asarray(results[core]["outT"]).T
    return out


_CACHE = {}


def _get_compiled(n_cores=8, ln_affine=True):
    key = ("nc", ln_affine)
    if key not in _CACHE:
        cfg = Cfg(C=1024, H=16, D=64, Tkv=2048)
        nc = bacc.Bacc("TRN2", target_bir_lowering=False, debug=False,
                       num_devices=n_cores)
        build_kernel(nc, cfg, ln_affine=ln_affine)
        nc.compile()
        _CACHE[key] = (nc, cfg)
    return _CACHE[key]


def kernel(**inputs):
    """Full transformer block on 8 NeuronCores. Takes the full unsharded
    inputs (as in reference.setup_inputs) and returns the full [4, 2048,
    1024] float32 output."""
    ln_affine = not (
        np.all(np.asarray(inputs["ln1_g"]) == 1)
        and np.all(np.asarray(inputs["ln1_b"]) == 0)
        and np.all(np.asarray(inputs["ln2_g"]) == 1)
        and np.all(np.asarray(inputs["ln2_b"]) == 0))
    nc, cfg = _get_compiled(8, ln_affine)
    x = np.asarray(inputs["x"])
    B = x.shape[0]
    n_cores = 2 * B
    shared = _shared_inputs(cfg, inputs)
    in_maps = [host_prepare(cfg, inputs, core, shared)
               for core in range(n_cores)]
    res = run_bass_kernel_spmd(nc, in_maps, core_ids=list(range(n_cores)))
    return host_gather(cfg, res.results, B)


# revision 38
# speedup vs baseline: 2.3376x; 2.3376x over previous
"""TRN2 Bass/Tile kernel for nn_Block_89842125898023 (dense transformer
block), SPMD over 8 NeuronCores.

Sharding (data-parallel over batch x query-halves, zero collectives):
core c handles batch element b = c//2 and query half p = c%2 of that
element's 2048 tokens, using a "zigzag" split (p=0: tokens [0,512) u
[1536,2048); p=1: [512,1536)) so the causal-attention work is identical
on every core. Each core redundantly computes K/V for its batch
element's full sequence from the (replicated) xT input.

v2 layout notes (vs the earlier DRAM-bounce version):
- K/Q/V and the attention output stay SBUF-resident end to end; the
  only DRAM traffic is inputs, weights and the final output.
- All weights are bf16 and streamed with plain contiguous HWDGE DMAs
  ([128, >=512] tiles); no software-DGE rearranged gathers.
- proj/FFN run once over both query halves (1024 columns), so Wp/W1/W2
  are each streamed exactly once per core.
- Causal masking is applied multiplicatively post-exp on the gpsimd
  engine (DVE is loaded with LN/eviction work).
- On-device layout is feature-major ([C, T], channels on partitions);
  scores are computed transposed per head; the softmax normalizer comes
  from a ones-column appended to V (M=65 AV matmul); division by Z is
  deferred to the AV eviction. All matmul accumulation is fp32 PSUM;
  residuals are carried in f32r, matmul operands in bf16.

kernel(**inputs) takes the full unsharded inputs, builds per-core input
maps host-side, runs the SPMD program on cores 0-7 via
bass_utils.run_bass_kernel_spmd, and reassembles the full output.
"""

import sys

sys.path.insert(0, "/opt/trn_rl_repo")

from contextlib import ExitStack

import numpy as np
import ml_dtypes

import concourse.bass as bass
import concourse.bacc as bacc
import concourse.tile as tile
from concourse import mybir
from concourse.bass_utils import run_bass_kernel_spmd

F32 = mybir.dt.float32
F32R = mybir.dt.float32r
BF16 = mybir.dt.bfloat16
AF = mybir.ActivationFunctionType
ALU = mybir.AluOpType
P = 128


class Cfg:
    def __init__(self, C=1024, H=16, D=64, Tkv=2048, eps=1e-5, ffn_mult=4):
        self.C = C
        self.H = H
        self.D = D
        assert H * D == C
        self.Tkv = Tkv
        self.Tq = Tkv // 2
        self.F = ffn_mult * C
        self.eps = eps
        self.NC = C // 128
        self.NF = self.F // 128
        self.NS = Tkv // 128
        self.scale = C ** -0.5
        self.TH = self.Tq // 2
        NS2 = self.NS // 2
        self.MB = 128 * (NS2 - 1) + self.TH
        self.MLO = ((Tkv - 128) - 128 * (NS2 - 1), 0)


def build_kernel(nc: bass.Bass, cfg: Cfg, ln_affine=True):
    c = cfg
    NH = c.C // 64  # head-halves
    TH = c.TH
    NS2 = c.NS // 2
    NTQ = c.Tq // 512  # 512-col chunks over the query axis

    # packed inputs: few, large tensors (per-call dispatch cost through
    # the axon tunnel scales with argument count)
    # xin = [xT | xqTb] bf16; wpack = all weights as 128-row blocks laid
    # out along one row-block's columns: wqkv | wp | w1 | w2
    xin_d = nc.dram_tensor("xin", [c.C, c.Tkv + c.Tq], BF16,
                           kind="ExternalInput")
    xqT_d = nc.dram_tensor("xqT", [c.C, c.Tq], F32R, kind="ExternalInput")
    WQKV0 = 0
    WP0 = WQKV0 + c.NC * 3 * c.C
    W10 = WP0 + c.NC * c.C
    W20 = W10 + c.NC * c.F
    WEND = W20 + c.NF * c.C
    wpack_d = nc.dram_tensor("wpack", [P, WEND], BF16, kind="ExternalInput")
    NV = 6 * (c.C // P) + c.F // P
    vecs_d = nc.dram_tensor("vecs", [P, NV], F32, kind="ExternalInput")
    mask_d = nc.dram_tensor("maskband", [P, 2 * c.MB], BF16,
                            kind="ExternalInput")
    out_d = nc.dram_tensor("outT", [c.C, c.Tq], F32, kind="ExternalOutput")

    with ExitStack() as ctx:
        tc = ctx.enter_context(tile.TileContext(nc))

        const_pool = ctx.enter_context(tc.tile_pool(name="const", bufs=1))
        ones_t = const_pool.tile([P, 1], F32)
        nc.vector.memset(ones_t[:], 1.0)
        zerob = const_pool.tile([P, 1], F32, name="zerob")
        nc.vector.memset(zerob[:], 0.0)
        epsb = const_pool.tile([1, 1], F32, name="epsb")
        nc.vector.memset(epsb[:], float(c.eps))
        ones_bf = const_pool.tile([P, 1], BF16, name="ones_bf")
        nc.vector.memset(ones_bf[:], 1.0)
        ones_r = const_pool.tile([P, 1], F32R, name="ones_r")
        nc.vector.tensor_copy(ones_r[:], ones_t[:])

        vec_tile = const_pool.tile([P, NV], F32, name="vecs")
        nc.sync.dma_start(out=vec_tile[:], in_=vecs_d.ap())
        _vo = [0]

        def vec_cols(n):
            k = n // P
            cols = [vec_tile[:, _vo[0] + i:_vo[0] + i + 1] for i in range(k)]
            _vo[0] += k
            return cols

        ln1g, ln1b = vec_cols(c.C), vec_cols(c.C)
        ln2g, ln2b = vec_cols(c.C), vec_cols(c.C)
        bp, b1, b2 = vec_cols(c.C), vec_cols(c.F), vec_cols(c.C)

        # attention output (concat heads, feature-major), both halves
        attn_pool = ctx.enter_context(tc.tile_pool(name="attn", bufs=1))
        attn_tiles = [attn_pool.tile([P, c.Tq], BF16, name=f"at{i}")
                      for i in range(c.NC)]

        with ExitStack() as pab:
            k_pool = pab.enter_context(tc.tile_pool(name="k", bufs=1))
            k_tiles = [k_pool.tile([P, c.Tkv], BF16, name=f"k{i}")
                       for i in range(c.NC)]
            q_pool = pab.enter_context(tc.tile_pool(name="q", bufs=1))
            q_tiles = [q_pool.tile([P, c.Tq], BF16, name=f"q{i}")
                       for i in range(c.NC)]
            v_pool = pab.enter_context(tc.tile_pool(name="v", bufs=1))
            v_tiles = [v_pool.tile([P, NH, 65], BF16, name=f"v{s}")
                       for s in range(c.NS)]

            # ---------- phase A+B: LN1, Q/K/V, attention ----------
            # V[8..15] is interleaved with w=0 attention per head so the
            # activation engine's exp work overlaps PE's V matmuls.
            with ExitStack() as pa:
                x_pool = pa.enter_context(tc.tile_pool(name="xT", bufs=1))
                xq_pool = pa.enter_context(tc.tile_pool(name="xq", bufs=1))
                w_pool = pa.enter_context(tc.tile_pool(name="wqkv", bufs=1))

                # x on the SP DMA queue, xq on the DVE queue: both input
                # streams flow concurrently at startup
                xq_tiles = []
                for ci in range(c.NC):
                    t = xq_pool.tile([P, c.Tq], BF16, name=f"xq{ci}")
                    nc.scalar.dma_start(
                        out=t[:],
                        in_=xin_d.ap()[ci * P:(ci + 1) * P,
                                       c.Tkv:c.Tkv + c.Tq])
                    xq_tiles.append(t)
                x_tiles = []
                for ci in range(c.NC):
                    t = x_pool.tile([P, c.Tkv], BF16, name=f"x{ci}")
                    nc.sync.dma_start(out=t[:],
                                      in_=xin_d.ap()[ci * P:(ci + 1) * P,
                                                     0:c.Tkv])
                    x_tiles.append(t)

                # LN1 in place (x tiles only feed the projections)
                _layernorm_fm(nc, tc, c, xq_tiles, xq_tiles, c.Tq,
                              ln1g if ln_affine else None, ln1b,
                              ones_bf, zerob, epsb, "ln1q")
                _layernorm_fm(nc, tc, c, x_tiles, x_tiles, c.Tkv,
                              ln1g if ln_affine else None, ln1b,
                              ones_bf, zerob, epsb, "ln1")

                def load_w(which):
                    ts = []
                    for ci in range(c.NC):
                        wt = w_pool.tile([P, c.C], BF16, name=f"w{ci}")
                        o = WQKV0 + ci * 3 * c.C + which * c.C
                        nc.scalar.dma_start(
                            out=wt[:], in_=wpack_d.ap()[:, o:o + c.C])
                        ts.append(wt)
                    return ts

                def v_block(s, psum_pool):
                    nc.vector.memset(v_tiles[s][:, :, 64:65], 1.0)
                    psv = [psum_pool.tile([P, 512], F32, name=f"pp{nf}")
                           for nf in range(2)]
                    for ci in range(c.NC):
                        for nf in range(2):
                            nc.tensor.matmul(
                                psv[nf][:],
                                lhsT=x_tiles[ci][:, s * P:(s + 1) * P],
                                rhs=wv_tiles[ci][:, nf * 512:(nf + 1) * 512],
                                start=(ci == 0), stop=(ci == c.NC - 1))
                    for nf in range(2):
                        nc.vector.tensor_copy(
                            v_tiles[s][:, nf * 8:(nf + 1) * 8, 0:64],
                            psv[nf][:].rearrange("p (h d) -> p h d", d=64))

                with ExitStack() as pq:
                    mm_psum = pq.enter_context(
                        tc.tile_pool(name="mm_psum", bufs=2, space="PSUM"))
                    # Q projection (query tokens only)
                    wq_tiles = load_w(0)
                    for fi in range(c.NC):
                        pss = [mm_psum.tile([P, 512], F32, name=f"pp{t}")
                               for t in range(NTQ)]
                        for ci in range(c.NC):
                            for t in range(NTQ):
                                nc.tensor.matmul(
                                    pss[t][:],
                                    lhsT=wq_tiles[ci][:,
                                                      fi * P:(fi + 1) * P],
                                    rhs=xq_tiles[ci][:,
                                                     t * 512:(t + 1) * 512],
                                    start=(ci == 0), stop=(ci == c.NC - 1))
                        for t in range(NTQ):
                            nc.vector.tensor_copy(
                                q_tiles[fi][:, t * 512:(t + 1) * 512],
                                pss[t][:])

                    # K projection (full sequence)
                    wk_tiles = load_w(1)
                    NTK = c.Tkv // 512
                    for fi in range(c.NC):
                        for t0 in range(0, NTK, 2):
                            pss = [mm_psum.tile([P, 512], F32,
                                                name=f"pp{t - t0}")
                                   for t in range(t0, t0 + 2)]
                            for ci in range(c.NC):
                                for i, t in enumerate(range(t0, t0 + 2)):
                                    nc.tensor.matmul(
                                        pss[i][:],
                                        lhsT=wk_tiles[ci][:,
                                                          fi * P:
                                                          (fi + 1) * P],
                                        rhs=x_tiles[ci][:,
                                                        t * 512:
                                                        (t + 1) * 512],
                                        start=(ci == 0),
                                        stop=(ci == c.NC - 1))
                            for i, t in enumerate(range(t0, t0 + 2)):
                                nc.vector.tensor_copy(
                                    k_tiles[fi][:, t * 512:(t + 1) * 512],
                                    pss[i][:])

                    # V projection, first half of the key blocks
                    wv_tiles = load_w(2)
                    for s in range(NS2):
                        v_block(s, mm_psum)

                # attention pools (mm_psum freed: 2+4+2 = 8 PSUM banks)
                sc_psum = pa.enter_context(
                    tc.tile_pool(name="sc_psum", bufs=2, space="PSUM"))
                av_psum = pa.enter_context(
                    tc.tile_pool(name="av_psum", bufs=1, space="PSUM"))
                vb_psum = pa.enter_context(
                    tc.tile_pool(name="vb_psum", bufs=1, space="PSUM"))
                e_pool = pa.enter_context(tc.tile_pool(name="e", bufs=3))
                r_pool = pa.enter_context(tc.tile_pool(name="r", bufs=2))
                mk_pool = pa.enter_context(tc.tile_pool(name="mk", bufs=1))

                mask_t = mk_pool.tile([P, 2 * c.MB], BF16, name="maskband")
                nc.scalar.dma_start(out=mask_t[:], in_=mask_d.ap())

                def attn_head(w, hp):
                    wsl = slice(w * TH, (w + 1) * TH)
                    avs = [av_psum.tile([65, TH], F32, name=f"av{half}")
                           for half in range(2)]
                    NJ = NS2 if w == 0 else c.NS
                    # j in pairs: scores for (j, j+1) land in the two
                    # halves of one 2-bank PSUM tile, one Exp covers both
                    for j0 in range(0, NJ, 2):
                        for half in range(2):
                            hsl = slice(half * 64, half * 64 + 64)
                            head = 2 * hp + half
                            ps = sc_psum.tile([P, 2 * TH], F32,
                                              name="ps_sc")
                            for i, j in enumerate((j0, j0 + 1)):
                                nc.tensor.matmul(
                                    ps[:, i * TH:(i + 1) * TH],
                                    lhsT=k_tiles[hp][hsl,
                                                     j * P:(j + 1) * P],
                                    rhs=q_tiles[hp][hsl, wsl],
                                    start=True, stop=True)
                            et = e_pool.tile([P, 2 * TH], BF16, name="et")
                            nc.scalar.activation(et[:], ps[:], AF.Exp,
                                                 bias=zerob[:],
                                                 scale=float(c.scale))
                            for i, j in enumerate((j0, j0 + 1)):
                                if w == 0 or j >= NS2:
                                    cj = (w * c.MB + (c.Tkv - 128)
                                          - 128 * j - c.MLO[w])
                                    esl = slice(i * TH, (i + 1) * TH)
                                    nc.vector.tensor_tensor(
                                        et[:, esl], et[:, esl],
                                        mask_t[:, cj: cj + TH],
                                        op=ALU.mult)
                            for i, j in enumerate((j0, j0 + 1)):
                                nc.tensor.matmul(
                                    avs[half][:],
                                    lhsT=v_tiles[j][:, head, :],
                                    rhs=et[:, i * TH:(i + 1) * TH],
                                    start=(j == 0),
                                    stop=(j == NJ - 1))
                    for half in range(2):
                        av = avs[half]
                        rt0 = r_pool.tile([1, TH], F32, name="rt0")
                        nc.vector.reciprocal(rt0[:], av[64:65, :])
                        rb = r_pool.tile([64, TH], F32, name="rb")
                        nc.gpsimd.partition_broadcast(rb[:], rt0[:])
                        if half == 0:
                            nc.vector.tensor_tensor(
                                attn_tiles[hp][0:64, wsl],
                                av[0:64, :], rb[:], op=ALU.mult)
                        else:
                            # engines can't shift partitions; bounce
                            # through SBUF->SBUF DMA to rows 64-127
                            ab = r_pool.tile([64, TH], BF16, name="ab")
                            nc.vector.tensor_tensor(
                                ab[:], av[0:64, :], rb[:], op=ALU.mult)
                            nc.sync.dma_start(
                                out=attn_tiles[hp][64:128, wsl],
                                in_=ab[:])

                # w=0 attention only needs v[0..NS2); interleave the
                # remaining V blocks with it to keep the PE fed while
                # the activation engine chews the exps
                for hp in range(c.NC):
                    v_block(NS2 + hp, vb_psum)
                    attn_head(0, hp)
                for hp in range(c.NC):
                    attn_head(1, hp)

        # ---------- phase C: proj + residual (both halves) ----------
        with ExitStack() as pcd:
            x1_pool = pcd.enter_context(tc.tile_pool(name="x1", bufs=1))
            x1_tiles = [x1_pool.tile([P, c.Tq], F32R, name=f"x1{i}")
                        for i in range(c.NC)]
            xs_pool = pcd.enter_context(tc.tile_pool(name="xs", bufs=2))
            with ExitStack() as pc:
                wp_pool = pc.enter_context(tc.tile_pool(name="wp", bufs=1))
                pj_psum = pc.enter_context(
                    tc.tile_pool(name="pj_psum", bufs=1, space="PSUM"))

                wp_tiles = []
                for ci in range(c.NC):
                    wt = wp_pool.tile([P, c.C], BF16, name=f"wp{ci}")
                    nc.sync.dma_start(
                        out=wt[:],
                        in_=wpack_d.ap()[:, WP0 + ci * c.C:
                                         WP0 + (ci + 1) * c.C])
                    wp_tiles.append(wt)

                for fg0 in range(0, c.NC, 4):
                    fis = range(fg0, fg0 + 4)
                    pss = {(fi, t): pj_psum.tile(
                        [P, 512], F32, name=f"pj{fi - fg0}_{t}")
                        for fi in fis for t in range(NTQ)}
                    for ci in range(c.NC):
                        for fi in fis:
                            for t in range(NTQ):
                                nc.tensor.matmul(
                                    pss[(fi, t)][:],
                                    lhsT=wp_tiles[ci][:, fi * P:(fi + 1) * P],
                                    rhs=attn_tiles[ci][:,
                                                       t * 512:(t + 1) * 512],
                                    start=(ci == 0), stop=(ci == c.NC - 1))
                    for fi in fis:
                        xqs = xs_pool.tile([P, c.Tq], F32R, name="xqs")
                        nc.sync.dma_start(
                            out=xqs[:], in_=xqT_d.ap()[fi * P:(fi + 1) * P, :])
                        for t in range(NTQ):
                            tsl = slice(t * 512, (t + 1) * 512)
                            ev = xs_pool.tile([P, 512], F32, name="ev")
                            nc.vector.tensor_scalar(ev[:], pss[(fi, t)][:],
                                                    bp[fi][:], None,
                                                    op0=ALU.add)
                            nc.vector.tensor_tensor(
                                x1_tiles[fi][:, tsl], ev[:], xqs[:, tsl],
                                op=ALU.add)

            # ---------- phase D: LN2 + FFN (both halves) ----------
            with ExitStack() as pd:
                h2_pool = pd.enter_context(tc.tile_pool(name="h2", bufs=1))
                h2_tiles = [h2_pool.tile([P, c.Tq], BF16, name=f"h2_{i}")
                            for i in range(c.NC)]
                _layernorm_fm(nc, tc, c, x1_tiles, h2_tiles, c.Tq,
                              ln2g if ln_affine else None, ln2b,
                              ones_r, zerob, epsb, "ln2")

                relu_pool = pd.enter_context(tc.tile_pool(name="relu",
                                                          bufs=1))
                relu_tiles = [relu_pool.tile([P, c.Tq], BF16, name=f"r{i}")
                              for i in range(c.NF)]

                # FFN W1: stream w1 once, both halves as rhs columns
                with ExitStack() as pw1:
                    w1_pool = pw1.enter_context(
                        tc.tile_pool(name="w1s", bufs=2))
                    ff_psum = pw1.enter_context(
                        tc.tile_pool(name="ff_psum", bufs=2, space="PSUM"))
                    for fg in range(c.F // c.C):
                        w1_tiles = []
                        for ci in range(c.NC):
                            wt = w1_pool.tile([P, c.C], BF16,
                                              name=f"w1s{ci}")
                            o = W10 + ci * c.F + fg * c.C
                            nc.sync.dma_start(
                                out=wt[:],
                                in_=wpack_d.ap()[:, o:o + c.C])
                            w1_tiles.append(wt)
                        for fsub in range(c.NC):
                            f = fg * c.NC + fsub
                            psw = [ff_psum.tile([P, 512], F32,
                                                name=f"pw{t}")
                                   for t in range(NTQ)]
                            for ci in range(c.NC):
                                for t in range(NTQ):
                                    nc.tensor.matmul(
                                        psw[t][:],
                                        lhsT=w1_tiles[ci][:,
                                                          fsub * P:
                                                          (fsub + 1) * P],
                                        rhs=h2_tiles[ci][:,
                                                         t * 512:
                                                         (t + 1) * 512],
                                        start=(ci == 0),
                                        stop=(ci == c.NC - 1))
                            for t in range(NTQ):
                                nc.scalar.activation(
                                    relu_tiles[f][:, t * 512:(t + 1) * 512],
                                    psw[t][:], AF.Relu, bias=b1[f][:])

                # FFN W2 + residual + out: fi-halves so PSUM fits
                with ExitStack() as pw2:
                    w2_pool = pw2.enter_context(
                        tc.tile_pool(name="w2s", bufs=4))
                    w2_psum = pw2.enter_context(
                        tc.tile_pool(name="w2_psum", bufs=1, space="PSUM"))
                    for fih in range(2):
                        fis = range(fih * 4, fih * 4 + 4)
                        pss = {(fi, t): w2_psum.tile(
                            [P, 512], F32, name=f"p2{fi % 4}_{t}")
                            for fi in fis for t in range(NTQ)}
                        for fb in range(c.NF):
                            wt = w2_pool.tile([P, 512], BF16, name="w2t")
                            o = W20 + fb * c.C + fih * 512
                            nc.sync.dma_start(
                                out=wt[:],
                                in_=wpack_d.ap()[:, o:o + 512])
                            for fi in fis:
                                fo = (fi - fih * 4) * P
                                for t in range(NTQ):
                                    nc.tensor.matmul(
                                        pss[(fi, t)][:],
                                        lhsT=wt[:, fo:fo + P],
                                        rhs=relu_tiles[fb][:,
                                                           t * 512:
                                                           (t + 1) * 512],
                                        start=(fb == 0),
                                        stop=(fb == c.NF - 1))
                        for fi in fis:
                            for t in range(NTQ):
                                tsl = slice(t * 512, (t + 1) * 512)
                                # bias-add on act (idle here), residual
                                # add on DVE: splits the eviction tail
                                ev = xs_pool.tile([P, 512], F32, name="ev2")
                                nc.scalar.activation(
                                    ev[:], pss[(fi, t)][:], AF.Identity,
                                    bias=b2[fi][:])
                                nc.vector.tensor_tensor(
                                    ev[:], ev[:], x1_tiles[fi][:, tsl],
                                    op=ALU.add)
                                nc.sync.dma_start(
                                    out=out_d.ap()[fi * P:(fi + 1) * P, tsl],
                                    in_=ev[:])
    return nc


def _layernorm_fm(nc, tc, c, x_tiles, out_tiles, T, g_tiles, b_tiles,
                  ones_t, zerob, epsb, name):
    """Feature-major layernorm: features on partitions, stats via
    ones-vector matmuls on the PE. Stats + apply are fused per 512-col
    chunk so row temporaries stay chunk-sized. In-place (out == x) is
    safe: chunk tt's stats read x[:, tt] strictly before its apply
    writes it, and other chunks touch disjoint columns."""
    with ExitStack() as ctx:
        TT = min(512, T)
        NT = T // TT
        sq_pool = ctx.enter_context(tc.tile_pool(name=f"{name}_sq", bufs=2))
        st_psum = ctx.enter_context(
            tc.tile_pool(name=f"{name}_stp", bufs=2, space="PSUM"))
        tmp_pool = ctx.enter_context(tc.tile_pool(name=f"{name}_tmp",
                                                  bufs=2))

        bf_in = x_tiles[0].dtype == BF16 and out_tiles[0].dtype == BF16
        bc_dt = BF16 if bf_in else F32
        # pass 1: stats for every chunk (PE runs the ones-matmuls
        # back-to-back; the act/DVE scalar chains trail behind)
        rows = []
        for tt in range(NT):
            sl = slice(tt * TT, (tt + 1) * TT)
            ps1 = st_psum.tile([1, TT], F32, name="ps1")
            ps2 = st_psum.tile([1, TT], F32, name="ps2")
            sq_dt = x_tiles[0].dtype
            for ci, xt in enumerate(x_tiles):
                st, sp = ci == 0, ci == len(x_tiles) - 1
                nc.tensor.matmul(ps1[:], lhsT=ones_t[:],
                                 rhs=xt[:, sl], start=st, stop=sp)
                sq = sq_pool.tile([P, TT], sq_dt, name="sq")
                if sq_dt == BF16:
                    nc.vector.tensor_tensor(sq[:], xt[:, sl], xt[:, sl],
                                            op=ALU.mult)
                else:
                    nc.scalar.activation(sq[:], xt[:, sl], AF.Square,
                                         bias=zerob[:])
                nc.tensor.matmul(ps2[:], lhsT=ones_t[:], rhs=sq[:],
                                 start=st, stop=sp)
            mu = tmp_pool.tile([1, TT], F32, name="mu")
            nc.scalar.mul(mu[:], ps1[:], 1.0 / c.C)
            mu2 = tmp_pool.tile([1, TT], F32, name="mu2")
            nc.scalar.activation(mu2[:], mu[:], AF.Square, bias=zerob[0:1])
            var = tmp_pool.tile([1, TT], F32, name="var")
            nc.scalar.mul(var[:], ps2[:], 1.0 / c.C)
            nc.vector.tensor_sub(var[:], var[:], mu2[:])
            sd = tmp_pool.tile([1, TT], F32, name="sd")
            nc.scalar.activation(sd[:], var[:], AF.Sqrt, bias=epsb[:])
            rs = tmp_pool.tile([1, TT], F32, name="rs", bufs=NT)
            nc.vector.reciprocal(rs[:], sd[:])
            nmrs = tmp_pool.tile([1, TT], F32, name="nmrs", bufs=NT)
            nc.vector.tensor_tensor(nmrs[:], mu[:], rs[:], op=ALU.mult)
            nc.vector.tensor_scalar_mul(nmrs[:], nmrs[:], -1.0)
            if bf_in:
                rrow = tmp_pool.tile([1, TT], BF16, name="rrowb", bufs=NT)
                nc.vector.tensor_copy(rrow[:], rs[:])
                nrow = tmp_pool.tile([1, TT], BF16, name="nrowb", bufs=NT)
                nc.vector.tensor_copy(nrow[:], nmrs[:])
            else:
                rrow, nrow = rs, nmrs
            rows.append((rrow, nrow))
        # pass 2: broadcast + apply per chunk
        for tt in range(NT):
            sl = slice(tt * TT, (tt + 1) * TT)
            rrow, nrow = rows[tt]
            rs_b = tmp_pool.tile([P, TT], bc_dt, name="rsb")
            nmrs_b = tmp_pool.tile([P, TT], bc_dt, name="nmrsb")
            nc.gpsimd.partition_broadcast(rs_b[:], rrow[:])
            nc.gpsimd.partition_broadcast(nmrs_b[:], nrow[:])
            for ci, (xt, ot) in enumerate(zip(x_tiles, out_tiles)):
                nc.vector.tensor_tensor(ot[:, sl], xt[:, sl], rs_b[:],
                                        op=ALU.mult)
                nc.vector.tensor_tensor(ot[:, sl], ot[:, sl], nmrs_b[:],
                                        op=ALU.add)
                if g_tiles is not None:
                    nc.vector.tensor_scalar(ot[:, sl], ot[:, sl],
                                            g_tiles[ci][:], b_tiles[ci][:],
                                            op0=ALU.mult, op1=ALU.add)


# ======================= host side =======================

def round_f32r(x):
    u = np.asarray(x, np.float32).view(np.uint32).astype(np.uint64)
    lsb = (u >> np.uint64(12)) & np.uint64(1)
    u = u + (np.uint64(1) << np.uint64(11)) - np.uint64(1) + lsb
    u = (u >> np.uint64(12)) << np.uint64(12)
    return (u & np.uint64(0xFFFFFFFF)).astype(np.uint32).view(np.float32)


def zigzag_tokens(cfg, p):
    c = cfg
    if p == 0:
        return np.concatenate([np.arange(0, c.TH),
                               np.arange(c.Tkv - c.TH, c.Tkv)])
    return np.arange(c.TH, c.TH + c.Tq)


def _shared_inputs(cfg, inputs):
    """Per-call (not per-core) tensors: weights, vecs, per-p masks."""
    c = cfg
    bf = lambda a: np.ascontiguousarray(
        np.asarray(a).astype(ml_dtypes.bfloat16))

    def flat_w(w):
        return np.transpose(np.asarray(w), (1, 0, 2)).reshape(c.C, c.C)

    wqkv = np.concatenate([flat_w(inputs["Wq"]), flat_w(inputs["Wk"]),
                           flat_w(inputs["Wv"])], axis=1)
    # wpack: every weight as its 128-row blocks, blocks' columns
    # concatenated along one [128, *] row: wqkv | wp | w1 | w2
    blocks = []
    for wmat in (wqkv, np.asarray(inputs["Wp"]), np.asarray(inputs["W1"]),
                 np.asarray(inputs["W2"])):
        blocks.append(np.asarray(wmat, np.float32)
                      .reshape(-1, P, wmat.shape[1])
                      .transpose(1, 0, 2).reshape(P, -1))
    wpack = bf(np.concatenate(blocks, axis=1))

    vl = []
    for nm in ("ln1_g", "ln1_b", "ln2_g", "ln2_b", "bp", "b1", "b2"):
        v = np.asarray(inputs[nm], np.float32)
        vl.append(v.reshape(-1, 128).T)
    vecs = np.ascontiguousarray(np.concatenate(vl, axis=1))

    g = np.arange(c.MB)[None, :]
    pp = np.arange(128)[:, None]
    masks = []
    for p in range(2):
        qoff_w = ((0, c.Tkv - c.TH), (c.TH, c.Tq))[p]
        bands = [(g + c.MLO[w] >= pp + (c.Tkv - 128) - qoff_w[w])
                 .astype(np.float32) for w in range(2)]
        masks.append(bf(np.concatenate(bands, axis=1)))

    xTs = []
    for b in range(np.asarray(inputs["x"]).shape[0]):
        xTs.append(np.asarray(inputs["x"])[b].T.astype(ml_dtypes.bfloat16))
    return dict(wpack=wpack, vecs=vecs, masks=masks, xTs=xTs)


def host_prepare(cfg, inputs, core_id, shared):
    c = cfg
    b, p = core_id // 2, core_id % 2
    xb = np.asarray(inputs["x"])[b]
    qidx = zigzag_tokens(c, p)
    xq = np.ascontiguousarray(xb[qidx].T)
    xin = np.concatenate(
        [shared["xTs"][b], xq.astype(ml_dtypes.bfloat16)], axis=1)
    return {
        "xin": np.ascontiguousarray(xin),
        "xqT": round_f32r(xq),
        "wpack": shared["wpack"],
        "vecs": shared["vecs"],
        "maskband": shared["masks"][p],
    }


def host_gather(cfg, results, B):
    c = cfg
    out = np.empty((B, c.Tkv, c.C), np.float32)
    for core in range(2 * B):
        b, p = core // 2, core % 2
        out[b, zigzag_tokens(c, p), :] = np.asarray(results[core]["outT"]).T
    return out


_CACHE = {}


def _get_compiled(n_cores=8, ln_affine=True):
    key = ("nc", ln_affine)
    if key not in _CACHE:
        cfg = Cfg(C=1024, H=16, D=64, Tkv=2048)
        nc = bacc.Bacc("TRN2", target_bir_lowering=False, debug=False,
                       num_devices=n_cores)
        build_kernel(nc, cfg, ln_affine=ln_affine)
        nc.compile()
        _CACHE[key] = (nc, cfg)
    return _CACHE[key]


def kernel(**inputs):
    """Full transformer block on 8 NeuronCores. Takes the full unsharded
    inputs (as in reference.setup_inputs) and returns the full [4, 2048,
    1024] float32 output."""
    ln_affine = not (
        np.all(np.asarray(inputs["ln1_g"]) == 1)
        and np.all(np.asarray(inputs["ln1_b"]) == 0)
        and np.all(np.asarray(inputs["ln2_g"]) == 1)
        and np.all(np.asarray(inputs["ln2_b"]) == 0))
    nc, cfg = _get_compiled(8, ln_affine)
    x = np.asarray(inputs["x"])
    B = x.shape[0]
    n_cores = 2 * B
    shared = _shared_inputs(cfg, inputs)
    in_maps = [host_prepare(cfg, inputs, core, shared)
               for core in range(n_cores)]
    res = run_bass_kernel_spmd(nc, in_maps, core_ids=list(range(n_cores)))
    return host_gather(cfg, res.results, B)


# revision 43
# speedup vs baseline: 2.4990x; 1.0690x over previous
"""TRN2 Bass/Tile kernel for nn_Block_89842125898023 (dense transformer
block), SPMD over 8 NeuronCores.

Sharding (data-parallel over batch x query-halves, zero collectives):
core c handles batch element b = c//2 and query half p = c%2 of that
element's 2048 tokens, using a "zigzag" split (p=0: tokens [0,512) u
[1536,2048); p=1: [512,1536)) so the causal-attention work is identical
on every core. Each core redundantly computes K/V for its batch
element's full sequence from the (replicated) xT input.

v2 layout notes (vs the earlier DRAM-bounce version):
- K/Q/V and the attention output stay SBUF-resident end to end; the
  only DRAM traffic is inputs, weights and the final output.
- All weights are bf16 and streamed with plain contiguous HWDGE DMAs
  ([128, >=512] tiles); no software-DGE rearranged gathers.
- proj/FFN run once over both query halves (1024 columns), so Wp/W1/W2
  are each streamed exactly once per core.
- Causal masking is applied multiplicatively post-exp on DVE (bf16
  SBUF operands hit the fast DVE modes); exp is batched over j-pairs
  into [128, 1024] two-bank PSUM reads to amortize the activation
  engine's fixed cost, and the second half of the V projection is
  interleaved with w=0 attention to keep the PE fed while the
  activation engine works through the exps.
- On-device layout is feature-major ([C, T], channels on partitions);
  scores are computed transposed per head; the softmax normalizer comes
  from a ones-column appended to V (M=65 AV matmul); division by Z is
  deferred to the AV eviction. All matmul accumulation is fp32 PSUM;
  residuals are carried in f32r, matmul operands in bf16.

kernel(**inputs) takes the full unsharded inputs, builds per-core input
maps host-side, runs the SPMD program on cores 0-7 via
bass_utils.run_bass_kernel_spmd, and reassembles the full output.
"""

import sys

sys.path.insert(0, "/opt/trn_rl_repo")

from contextlib import ExitStack

import numpy as np
import ml_dtypes

import concourse.bass as bass
import concourse.bacc as bacc
import concourse.tile as tile
from concourse import mybir
from concourse.bass_utils import run_bass_kernel_spmd

F32 = mybir.dt.float32
F32R = mybir.dt.float32r
BF16 = mybir.dt.bfloat16
AF = mybir.ActivationFunctionType
ALU = mybir.AluOpType
P = 128


class Cfg:
    def __init__(self, C=1024, H=16, D=64, Tkv=2048, eps=1e-5, ffn_mult=4):
        self.C = C
        self.H = H
        self.D = D
        assert H * D == C
        self.Tkv = Tkv
        self.Tq = Tkv // 2
        self.F = ffn_mult * C
        self.eps = eps
        self.NC = C // 128
        self.NF = self.F // 128
        self.NS = Tkv // 128
        self.scale = C ** -0.5
        self.TH = self.Tq // 2
        NS2 = self.NS // 2
        self.MB = 128 * (NS2 - 1) + self.TH
        self.MLO = ((Tkv - 128) - 128 * (NS2 - 1), 0)


def build_kernel(nc: bass.Bass, cfg: Cfg, ln_affine=True):
    c = cfg
    NH = c.C // 64  # head-halves
    TH = c.TH
    NS2 = c.NS // 2
    NTQ = c.Tq // 512  # 512-col chunks over the query axis

    # packed inputs: few, large tensors (per-call dispatch cost through
    # the axon tunnel scales with argument count)
    # xin = [xT | xqTb] bf16; wpack = all weights as 128-row blocks laid
    # out along one row-block's columns: wqkv | wp | w1 | w2
    xin_d = nc.dram_tensor("xin", [c.C, c.Tkv + c.Tq], BF16,
                           kind="ExternalInput")
    xqT_d = nc.dram_tensor("xqT", [c.C, c.Tq], F32R, kind="ExternalInput")
    WQKV0 = 0
    WP0 = WQKV0 + c.NC * 3 * c.C
    W10 = WP0 + c.NC * c.C
    W20 = W10 + c.NC * c.F
    MASK0 = W20 + c.NF * c.C
    WEND = MASK0 + 2 * c.MB
    wpack_d = nc.dram_tensor("wpack", [P, WEND], BF16, kind="ExternalInput")
    NV = 6 * (c.C // P) + c.F // P
    vecs_d = nc.dram_tensor("vecs", [P, NV], F32, kind="ExternalInput")
    out_d = nc.dram_tensor("outT", [c.C, c.Tq], F32, kind="ExternalOutput")

    with ExitStack() as ctx:
        tc = ctx.enter_context(tile.TileContext(nc))

        const_pool = ctx.enter_context(tc.tile_pool(name="const", bufs=1))
        ones_t = const_pool.tile([P, 1], F32)
        nc.vector.memset(ones_t[:], 1.0)
        zerob = const_pool.tile([P, 1], F32, name="zerob")
        nc.vector.memset(zerob[:], 0.0)
        epsb = const_pool.tile([1, 1], F32, name="epsb")
        nc.vector.memset(epsb[:], float(c.eps))
        ones_bf = const_pool.tile([P, 1], BF16, name="ones_bf")
        nc.vector.memset(ones_bf[:], 1.0)
        ones_r = const_pool.tile([P, 1], F32R, name="ones_r")
        nc.vector.tensor_copy(ones_r[:], ones_t[:])

        vec_tile = const_pool.tile([P, NV], F32, name="vecs")
        nc.sync.dma_start(out=vec_tile[:], in_=vecs_d.ap())
        _vo = [0]

        def vec_cols(n):
            k = n // P
            cols = [vec_tile[:, _vo[0] + i:_vo[0] + i + 1] for i in range(k)]
            _vo[0] += k
            return cols

        ln1g, ln1b = vec_cols(c.C), vec_cols(c.C)
        ln2g, ln2b = vec_cols(c.C), vec_cols(c.C)
        bp, b1, b2 = vec_cols(c.C), vec_cols(c.F), vec_cols(c.C)

        # attention output (concat heads, feature-major), both halves
        attn_pool = ctx.enter_context(tc.tile_pool(name="attn", bufs=1))
        attn_tiles = [attn_pool.tile([P, c.Tq], BF16, name=f"at{i}")
                      for i in range(c.NC)]

        with ExitStack() as pab:
            k_pool = pab.enter_context(tc.tile_pool(name="k", bufs=1))
            k_tiles = [k_pool.tile([P, c.Tkv], BF16, name=f"k{i}")
                       for i in range(c.NC)]
            q_pool = pab.enter_context(tc.tile_pool(name="q", bufs=1))
            q_tiles = [q_pool.tile([P, c.Tq], BF16, name=f"q{i}")
                       for i in range(c.NC)]
            v_pool = pab.enter_context(tc.tile_pool(name="v", bufs=1))
            v_tiles = [v_pool.tile([P, NH, 65], BF16, name=f"v{s}")
                       for s in range(c.NS)]

            # ---------- phase A+B: LN1, Q/K/V, attention ----------
            # V[8..15] is interleaved with w=0 attention per head so the
            # activation engine's exp work overlaps PE's V matmuls.
            with ExitStack() as pa:
                x_pool = pa.enter_context(tc.tile_pool(name="xT", bufs=1))
                xq_pool = pa.enter_context(tc.tile_pool(name="xq", bufs=1))
                w_pool = pa.enter_context(tc.tile_pool(name="wqkv", bufs=1))

                # x on the SP DMA queue, xq on the DVE queue: both input
                # streams flow concurrently at startup
                xq_tiles = []
                for ci in range(c.NC):
                    t = xq_pool.tile([P, c.Tq], BF16, name=f"xq{ci}")
                    nc.scalar.dma_start(
                        out=t[:],
                        in_=xin_d.ap()[ci * P:(ci + 1) * P,
                                       c.Tkv:c.Tkv + c.Tq])
                    xq_tiles.append(t)
                x_tiles = []
                for ci in range(c.NC):
                    t = x_pool.tile([P, c.Tkv], BF16, name=f"x{ci}")
                    nc.sync.dma_start(out=t[:],
                                      in_=xin_d.ap()[ci * P:(ci + 1) * P,
                                                     0:c.Tkv])
                    x_tiles.append(t)

                # LN1 in place (x tiles only feed the projections)
                _layernorm_fm(nc, tc, c, xq_tiles, xq_tiles, c.Tq,
                              ln1g if ln_affine else None, ln1b,
                              ones_bf, zerob, epsb, "ln1q")
                _layernorm_fm(nc, tc, c, x_tiles, x_tiles, c.Tkv,
                              ln1g if ln_affine else None, ln1b,
                              ones_bf, zerob, epsb, "ln1")

                def load_w(which):
                    ts = []
                    for ci in range(c.NC):
                        wt = w_pool.tile([P, c.C], BF16, name=f"w{ci}")
                        o = WQKV0 + ci * 3 * c.C + which * c.C
                        nc.scalar.dma_start(
                            out=wt[:], in_=wpack_d.ap()[:, o:o + c.C])
                        ts.append(wt)
                    return ts

                def v_block(s, psum_pool):
                    nc.vector.memset(v_tiles[s][:, :, 64:65], 1.0)
                    psv = [psum_pool.tile([P, 512], F32, name=f"pp{nf}")
                           for nf in range(2)]
                    for ci in range(c.NC):
                        for nf in range(2):
                            nc.tensor.matmul(
                                psv[nf][:],
                                lhsT=x_tiles[ci][:, s * P:(s + 1) * P],
                                rhs=wv_tiles[ci][:, nf * 512:(nf + 1) * 512],
                                start=(ci == 0), stop=(ci == c.NC - 1))
                    for nf in range(2):
                        nc.vector.tensor_copy(
                            v_tiles[s][:, nf * 8:(nf + 1) * 8, 0:64],
                            psv[nf][:].rearrange("p (h d) -> p h d", d=64))

                with ExitStack() as pq:
                    mm_psum = pq.enter_context(
                        tc.tile_pool(name="mm_psum", bufs=2, space="PSUM"))
                    # Q projection (query tokens only)
                    wq_tiles = load_w(0)
                    for fi in range(c.NC):
                        pss = [mm_psum.tile([P, 512], F32, name=f"pp{t}")
                               for t in range(NTQ)]
                        for ci in range(c.NC):
                            for t in range(NTQ):
                                nc.tensor.matmul(
                                    pss[t][:],
                                    lhsT=wq_tiles[ci][:,
                                                      fi * P:(fi + 1) * P],
                                    rhs=xq_tiles[ci][:,
                                                     t * 512:(t + 1) * 512],
                                    start=(ci == 0), stop=(ci == c.NC - 1))
                        for t in range(NTQ):
                            nc.vector.tensor_copy(
                                q_tiles[fi][:, t * 512:(t + 1) * 512],
                                pss[t][:])

                    # K projection (full sequence)
                    wk_tiles = load_w(1)
                    NTK = c.Tkv // 512
                    for fi in range(c.NC):
                        for t0 in range(0, NTK, 2):
                            pss = [mm_psum.tile([P, 512], F32,
                                                name=f"pp{t - t0}")
                                   for t in range(t0, t0 + 2)]
                            for ci in range(c.NC):
                                for i, t in enumerate(range(t0, t0 + 2)):
                                    nc.tensor.matmul(
                                        pss[i][:],
                                        lhsT=wk_tiles[ci][:,
                                                          fi * P:
                                                          (fi + 1) * P],
                                        rhs=x_tiles[ci][:,
                                                        t * 512:
                                                        (t + 1) * 512],
                                        start=(ci == 0),
                                        stop=(ci == c.NC - 1))
                            for i, t in enumerate(range(t0, t0 + 2)):
                                nc.vector.tensor_copy(
                                    k_tiles[fi][:, t * 512:(t + 1) * 512],
                                    pss[i][:])

                    # V projection, first half of the key blocks
                    wv_tiles = load_w(2)
                    for s in range(NS2):
                        v_block(s, mm_psum)

                # attention pools (mm_psum freed: 2+4+2 = 8 PSUM banks)
                sc_psum = pa.enter_context(
                    tc.tile_pool(name="sc_psum", bufs=2, space="PSUM"))
                av_psum = pa.enter_context(
                    tc.tile_pool(name="av_psum", bufs=1, space="PSUM"))
                vb_psum = pa.enter_context(
                    tc.tile_pool(name="vb_psum", bufs=1, space="PSUM"))
                e_pool = pa.enter_context(tc.tile_pool(name="e", bufs=3))
                r_pool = pa.enter_context(tc.tile_pool(name="r", bufs=2))
                mk_pool = pa.enter_context(tc.tile_pool(name="mk", bufs=1))

                mask_t = mk_pool.tile([P, 2 * c.MB], BF16, name="maskband")
                nc.scalar.dma_start(
                    out=mask_t[:],
                    in_=wpack_d.ap()[:, MASK0:MASK0 + 2 * c.MB])

                def attn_head(w, hp):
                    wsl = slice(w * TH, (w + 1) * TH)
                    avs = [av_psum.tile([65, TH], F32, name=f"av{half}")
                           for half in range(2)]
                    NJ = NS2 if w == 0 else c.NS
                    # j in pairs: scores for (j, j+1) land in the two
                    # halves of one 2-bank PSUM tile, one Exp covers both
                    for j0 in range(0, NJ, 2):
                        for half in range(2):
                            hsl = slice(half * 64, half * 64 + 64)
                            head = 2 * hp + half
                            ps = sc_psum.tile([P, 2 * TH], F32,
                                              name="ps_sc")
                            for i, j in enumerate((j0, j0 + 1)):
                                nc.tensor.matmul(
                                    ps[:, i * TH:(i + 1) * TH],
                                    lhsT=k_tiles[hp][hsl,
                                                     j * P:(j + 1) * P],
                                    rhs=q_tiles[hp][hsl, wsl],
                                    start=True, stop=True)
                            et = e_pool.tile([P, 2 * TH], BF16, name="et")
                            nc.scalar.activation(et[:], ps[:], AF.Exp,
                                                 bias=zerob[:],
                                                 scale=float(c.scale))
                            for i, j in enumerate((j0, j0 + 1)):
                                if w == 0 or j >= NS2:
                                    cj = (w * c.MB + (c.Tkv - 128)
                                          - 128 * j - c.MLO[w])
                                    esl = slice(i * TH, (i + 1) * TH)
                                    nc.vector.tensor_tensor(
                                        et[:, esl], et[:, esl],
                                        mask_t[:, cj: cj + TH],
                                        op=ALU.mult)
                            for i, j in enumerate((j0, j0 + 1)):
                                nc.tensor.matmul(
                                    avs[half][:],
                                    lhsT=v_tiles[j][:, head, :],
                                    rhs=et[:, i * TH:(i + 1) * TH],
                                    start=(j == 0),
                                    stop=(j == NJ - 1))
                    for half in range(2):
                        av = avs[half]
                        rt0 = r_pool.tile([1, TH], F32, name="rt0")
                        nc.vector.reciprocal(rt0[:], av[64:65, :])
                        rb = r_pool.tile([64, TH], F32, name="rb")
                        nc.gpsimd.partition_broadcast(rb[:], rt0[:])
                        if half == 0:
                            nc.vector.tensor_tensor(
                                attn_tiles[hp][0:64, wsl],
                                av[0:64, :], rb[:], op=ALU.mult)
                        else:
                            # engines can't shift partitions; bounce
                            # through SBUF->SBUF DMA to rows 64-127
                            ab = r_pool.tile([64, TH], BF16, name="ab")
                            nc.vector.tensor_tensor(
                                ab[:], av[0:64, :], rb[:], op=ALU.mult)
                            nc.sync.dma_start(
                                out=attn_tiles[hp][64:128, wsl],
                                in_=ab[:])

                # w=0 attention only needs v[0..NS2); interleave the
                # remaining V blocks with it to keep the PE fed while
                # the activation engine chews the exps
                for hp in range(c.NC):
                    v_block(NS2 + hp, vb_psum)
                    attn_head(0, hp)
                for hp in range(c.NC):
                    attn_head(1, hp)

        # ---------- phase C: proj + residual (both halves) ----------
        with ExitStack() as pcd:
            x1_pool = pcd.enter_context(tc.tile_pool(name="x1", bufs=1))
            x1_tiles = [x1_pool.tile([P, c.Tq], F32R, name=f"x1{i}")
                        for i in range(c.NC)]
            xs_pool = pcd.enter_context(tc.tile_pool(name="xs", bufs=2))
            with ExitStack() as pc:
                wp_pool = pc.enter_context(tc.tile_pool(name="wp", bufs=1))
                pj_psum = pc.enter_context(
                    tc.tile_pool(name="pj_psum", bufs=1, space="PSUM"))

                wp_tiles = []
                for ci in range(c.NC):
                    wt = wp_pool.tile([P, c.C], BF16, name=f"wp{ci}")
                    nc.sync.dma_start(
                        out=wt[:],
                        in_=wpack_d.ap()[:, WP0 + ci * c.C:
                                         WP0 + (ci + 1) * c.C])
                    wp_tiles.append(wt)

                for fg0 in range(0, c.NC, 4):
                    fis = range(fg0, fg0 + 4)
                    pss = {(fi, t): pj_psum.tile(
                        [P, 512], F32, name=f"pj{fi - fg0}_{t}")
                        for fi in fis for t in range(NTQ)}
                    for ci in range(c.NC):
                        for fi in fis:
                            for t in range(NTQ):
                                nc.tensor.matmul(
                                    pss[(fi, t)][:],
                                    lhsT=wp_tiles[ci][:, fi * P:(fi + 1) * P],
                                    rhs=attn_tiles[ci][:,
                                                       t * 512:(t + 1) * 512],
                                    start=(ci == 0), stop=(ci == c.NC - 1))
                    for fi in fis:
                        xqs = xs_pool.tile([P, c.Tq], F32R, name="xqs")
                        nc.sync.dma_start(
                            out=xqs[:], in_=xqT_d.ap()[fi * P:(fi + 1) * P, :])
                        for t in range(NTQ):
                            tsl = slice(t * 512, (t + 1) * 512)
                            ev = xs_pool.tile([P, 512], F32, name="ev")
                            nc.vector.tensor_scalar(ev[:], pss[(fi, t)][:],
                                                    bp[fi][:], None,
                                                    op0=ALU.add)
                            nc.vector.tensor_tensor(
                                x1_tiles[fi][:, tsl], ev[:], xqs[:, tsl],
                                op=ALU.add)

            # ---------- phase D: LN2 + FFN (both halves) ----------
            with ExitStack() as pd:
                h2_pool = pd.enter_context(tc.tile_pool(name="h2", bufs=1))
                h2_tiles = [h2_pool.tile([P, c.Tq], BF16, name=f"h2_{i}")
                            for i in range(c.NC)]
                _layernorm_fm(nc, tc, c, x1_tiles, h2_tiles, c.Tq,
                              ln2g if ln_affine else None, ln2b,
                              ones_r, zerob, epsb, "ln2")

                relu_pool = pd.enter_context(tc.tile_pool(name="relu",
                                                          bufs=1))
                relu_tiles = [relu_pool.tile([P, c.Tq], BF16, name=f"r{i}")
                              for i in range(c.NF)]

                # FFN W1: stream w1 once, both halves as rhs columns
                with ExitStack() as pw1:
                    w1_pool = pw1.enter_context(
                        tc.tile_pool(name="w1s", bufs=2))
                    ff_psum = pw1.enter_context(
                        tc.tile_pool(name="ff_psum", bufs=2, space="PSUM"))
                    for fg in range(c.F // c.C):
                        w1_tiles = []
                        for ci in range(c.NC):
                            wt = w1_pool.tile([P, c.C], BF16,
                                              name=f"w1s{ci}")
                            o = W10 + ci * c.F + fg * c.C
                            nc.sync.dma_start(
                                out=wt[:],
                                in_=wpack_d.ap()[:, o:o + c.C])
                            w1_tiles.append(wt)
                        for fsub in range(c.NC):
                            f = fg * c.NC + fsub
                            psw = [ff_psum.tile([P, 512], F32,
                                                name=f"pw{t}")
                                   for t in range(NTQ)]
                            for ci in range(c.NC):
                                for t in range(NTQ):
                                    nc.tensor.matmul(
                                        psw[t][:],
                                        lhsT=w1_tiles[ci][:,
                                                          fsub * P:
                                                          (fsub + 1) * P],
                                        rhs=h2_tiles[ci][:,
                                                         t * 512:
                                                         (t + 1) * 512],
                                        start=(ci == 0),
                                        stop=(ci == c.NC - 1))
                            for t in range(NTQ):
                                nc.scalar.activation(
                                    relu_tiles[f][:, t * 512:(t + 1) * 512],
                                    psw[t][:], AF.Relu, bias=b1[f][:])

                # FFN W2 + residual + out: fi-halves so PSUM fits
                with ExitStack() as pw2:
                    w2_pool = pw2.enter_context(
                        tc.tile_pool(name="w2s", bufs=4))
                    w2_psum = pw2.enter_context(
                        tc.tile_pool(name="w2_psum", bufs=1, space="PSUM"))
                    for fih in range(2):
                        fis = range(fih * 4, fih * 4 + 4)
                        pss = {(fi, t): w2_psum.tile(
                            [P, 512], F32, name=f"p2{fi % 4}_{t}")
                            for fi in fis for t in range(NTQ)}
                        for fb in range(c.NF):
                            wt = w2_pool.tile([P, 512], BF16, name="w2t")
                            o = W20 + fb * c.C + fih * 512
                            nc.sync.dma_start(
                                out=wt[:],
                                in_=wpack_d.ap()[:, o:o + 512])
                            for fi in fis:
                                fo = (fi - fih * 4) * P
                                for t in range(NTQ):
                                    nc.tensor.matmul(
                                        pss[(fi, t)][:],
                                        lhsT=wt[:, fo:fo + P],
                                        rhs=relu_tiles[fb][:,
                                                           t * 512:
                                                           (t + 1) * 512],
                                        start=(fb == 0),
                                        stop=(fb == c.NF - 1))
                        for fi in fis:
                            for t in range(NTQ):
                                tsl = slice(t * 512, (t + 1) * 512)
                                # bias-add on act (idle here), residual
                                # add on DVE: splits the eviction tail
                                ev = xs_pool.tile([P, 512], F32, name="ev2")
                                nc.scalar.activation(
                                    ev[:], pss[(fi, t)][:], AF.Identity,
                                    bias=b2[fi][:])
                                nc.vector.tensor_tensor(
                                    ev[:], ev[:], x1_tiles[fi][:, tsl],
                                    op=ALU.add)
                                nc.sync.dma_start(
                                    out=out_d.ap()[fi * P:(fi + 1) * P, tsl],
                                    in_=ev[:])
    return nc


def _layernorm_fm(nc, tc, c, x_tiles, out_tiles, T, g_tiles, b_tiles,
                  ones_t, zerob, epsb, name):
    """Feature-major layernorm: features on partitions, stats via
    ones-vector matmuls on the PE. Stats + apply are fused per 512-col
    chunk so row temporaries stay chunk-sized. In-place (out == x) is
    safe: chunk tt's stats read x[:, tt] strictly before its apply
    writes it, and other chunks touch disjoint columns."""
    with ExitStack() as ctx:
        TT = min(512, T)
        NT = T // TT
        sq_pool = ctx.enter_context(tc.tile_pool(name=f"{name}_sq", bufs=2))
        st_psum = ctx.enter_context(
            tc.tile_pool(name=f"{name}_stp", bufs=2, space="PSUM"))
        tmp_pool = ctx.enter_context(tc.tile_pool(name=f"{name}_tmp",
                                                  bufs=2))

        bf_in = x_tiles[0].dtype == BF16 and out_tiles[0].dtype == BF16
        bc_dt = BF16 if bf_in else F32
        # pass 1: stats for every chunk (PE runs the ones-matmuls
        # back-to-back; the act/DVE scalar chains trail behind)
        rows = []
        for tt in range(NT):
            sl = slice(tt * TT, (tt + 1) * TT)
            ps1 = st_psum.tile([1, TT], F32, name="ps1")
            ps2 = st_psum.tile([1, TT], F32, name="ps2")
            sq_dt = x_tiles[0].dtype
            for ci, xt in enumerate(x_tiles):
                st, sp = ci == 0, ci == len(x_tiles) - 1
                nc.tensor.matmul(ps1[:], lhsT=ones_t[:],
                                 rhs=xt[:, sl], start=st, stop=sp)
                sq = sq_pool.tile([P, TT], sq_dt, name="sq")
                if sq_dt == BF16:
                    nc.vector.tensor_tensor(sq[:], xt[:, sl], xt[:, sl],
                                            op=ALU.mult)
                else:
                    nc.scalar.activation(sq[:], xt[:, sl], AF.Square,
                                         bias=zerob[:])
                nc.tensor.matmul(ps2[:], lhsT=ones_t[:], rhs=sq[:],
                                 start=st, stop=sp)
            mu = tmp_pool.tile([1, TT], F32, name="mu")
            nc.scalar.mul(mu[:], ps1[:], 1.0 / c.C)
            mu2 = tmp_pool.tile([1, TT], F32, name="mu2")
            nc.scalar.activation(mu2[:], mu[:], AF.Square, bias=zerob[0:1])
            var = tmp_pool.tile([1, TT], F32, name="var")
            nc.scalar.mul(var[:], ps2[:], 1.0 / c.C)
            nc.vector.tensor_sub(var[:], var[:], mu2[:])
            sd = tmp_pool.tile([1, TT], F32, name="sd")
            nc.scalar.activation(sd[:], var[:], AF.Sqrt, bias=epsb[:])
            rs = tmp_pool.tile([1, TT], F32, name="rs", bufs=NT)
            nc.vector.reciprocal(rs[:], sd[:])
            nmrs = tmp_pool.tile([1, TT], F32, name="nmrs", bufs=NT)
            nc.vector.tensor_tensor(nmrs[:], mu[:], rs[:], op=ALU.mult)
            nc.vector.tensor_scalar_mul(nmrs[:], nmrs[:], -1.0)
            if bf_in:
                rrow = tmp_pool.tile([1, TT], BF16, name="rrowb", bufs=NT)
                nc.vector.tensor_copy(rrow[:], rs[:])
                nrow = tmp_pool.tile([1, TT], BF16, name="nrowb", bufs=NT)
                nc.vector.tensor_copy(nrow[:], nmrs[:])
            else:
                rrow, nrow = rs, nmrs
            rows.append((rrow, nrow))
        # pass 2: broadcast + apply per chunk
        for tt in range(NT):
            sl = slice(tt * TT, (tt + 1) * TT)
            rrow, nrow = rows[tt]
            rs_b = tmp_pool.tile([P, TT], bc_dt, name="rsb")
            nmrs_b = tmp_pool.tile([P, TT], bc_dt, name="nmrsb")
            nc.gpsimd.partition_broadcast(rs_b[:], rrow[:])
            nc.gpsimd.partition_broadcast(nmrs_b[:], nrow[:])
            for ci, (xt, ot) in enumerate(zip(x_tiles, out_tiles)):
                nc.vector.tensor_tensor(ot[:, sl], xt[:, sl], rs_b[:],
                                        op=ALU.mult)
                nc.vector.tensor_tensor(ot[:, sl], ot[:, sl], nmrs_b[:],
                                        op=ALU.add)
                if g_tiles is not None:
                    nc.vector.tensor_scalar(ot[:, sl], ot[:, sl],
                                            g_tiles[ci][:], b_tiles[ci][:],
                                            op0=ALU.mult, op1=ALU.add)


# ======================= host side =======================

def round_f32r(x):
    u = np.asarray(x, np.float32).view(np.uint32).astype(np.uint64)
    lsb = (u >> np.uint64(12)) & np.uint64(1)
    u = u + (np.uint64(1) << np.uint64(11)) - np.uint64(1) + lsb
    u = (u >> np.uint64(12)) << np.uint64(12)
    return (u & np.uint64(0xFFFFFFFF)).astype(np.uint32).view(np.float32)


def zigzag_tokens(cfg, p):
    c = cfg
    if p == 0:
        return np.concatenate([np.arange(0, c.TH),
                               np.arange(c.Tkv - c.TH, c.Tkv)])
    return np.arange(c.TH, c.TH + c.Tq)


def _shared_inputs(cfg, inputs):
    """Per-call (not per-core) tensors: weights, vecs, per-p masks."""
    c = cfg
    bf = lambda a: np.ascontiguousarray(
        np.asarray(a).astype(ml_dtypes.bfloat16))

    def flat_w(w):
        return np.transpose(np.asarray(w), (1, 0, 2)).reshape(c.C, c.C)

    wqkv = np.concatenate([flat_w(inputs["Wq"]), flat_w(inputs["Wk"]),
                           flat_w(inputs["Wv"])], axis=1)
    # wpack: every weight as its 128-row blocks, blocks' columns
    # concatenated along one [128, *] row: wqkv | wp | w1 | w2
    blocks = []
    for wmat in (wqkv, np.asarray(inputs["Wp"]), np.asarray(inputs["W1"]),
                 np.asarray(inputs["W2"])):
        blocks.append(np.asarray(wmat, np.float32)
                      .reshape(-1, P, wmat.shape[1])
                      .transpose(1, 0, 2).reshape(P, -1))
    wpack = bf(np.concatenate(blocks, axis=1))

    vl = []
    for nm in ("ln1_g", "ln1_b", "ln2_g", "ln2_b", "bp", "b1", "b2"):
        v = np.asarray(inputs[nm], np.float32)
        vl.append(v.reshape(-1, 128).T)
    vecs = np.ascontiguousarray(np.concatenate(vl, axis=1))

    g = np.arange(c.MB)[None, :]
    pp = np.arange(128)[:, None]
    wpacks = []
    for p in range(2):
        qoff_w = ((0, c.Tkv - c.TH), (c.TH, c.Tq))[p]
        bands = [(g + c.MLO[w] >= pp + (c.Tkv - 128) - qoff_w[w])
                 .astype(np.float32) for w in range(2)]
        mask = bf(np.concatenate(bands, axis=1))
        # mask band appended to the weight pack (one fewer dispatch arg)
        wpacks.append(np.ascontiguousarray(
            np.concatenate([wpack, mask], axis=1)))

    xTs = []
    for b in range(np.asarray(inputs["x"]).shape[0]):
        xTs.append(np.asarray(inputs["x"])[b].T.astype(ml_dtypes.bfloat16))
    return dict(wpacks=wpacks, vecs=vecs, xTs=xTs)


def host_prepare(cfg, inputs, core_id, shared):
    c = cfg
    b, p = core_id // 2, core_id % 2
    xb = np.asarray(inputs["x"])[b]
    qidx = zigzag_tokens(c, p)
    xq = np.ascontiguousarray(xb[qidx].T)
    xin = np.concatenate(
        [shared["xTs"][b], xq.astype(ml_dtypes.bfloat16)], axis=1)
    return {
        "xin": np.ascontiguousarray(xin),
        "xqT": round_f32r(xq),
        "wpack": shared["wpacks"][p],
        "vecs": shared["vecs"],
    }


def host_gather(cfg, results, B):
    c = cfg
    out = np.empty((B, c.Tkv, c.C), np.float32)
    for core in range(2 * B):
        b, p = core // 2, core % 2
        out[b, zigzag_tokens(c, p), :] = np.asarray(results[core]["outT"]).T
    return out


_CACHE = {}


def _get_compiled(n_cores=8, ln_affine=True):
    key = ("nc", ln_affine)
    if key not in _CACHE:
        cfg = Cfg(C=1024, H=16, D=64, Tkv=2048)
        nc = bacc.Bacc("TRN2", target_bir_lowering=False, debug=False,
                       num_devices=n_cores)
        build_kernel(nc, cfg, ln_affine=ln_affine)
        nc.compile()
        _CACHE[key] = (nc, cfg)
    return _CACHE[key]


def kernel(**inputs):
    """Full transformer block on 8 NeuronCores. Takes the full unsharded
    inputs (as in reference.setup_inputs) and returns the full [4, 2048,
    1024] float32 output."""
    ln_affine = not (
        np.all(np.asarray(inputs["ln1_g"]) == 1)
        and np.all(np.asarray(inputs["ln1_b"]) == 0)
        and np.all(np.asarray(inputs["ln2_g"]) == 1)
        and np.all(np.asarray(inputs["ln2_b"]) == 0))
    nc, cfg = _get_compiled(8, ln_affine)
    x = np.asarray(inputs["x"])
    B = x.shape[0]
    n_cores = 2 * B
    shared = _shared_inputs(cfg, inputs)
    in_maps = [host_prepare(cfg, inputs, core, shared)
               for core in range(n_cores)]
    res = run_bass_kernel_spmd(nc, in_maps, core_ids=list(range(n_cores)))
    return host_gather(cfg, res.results, B)
